# revision 29
# baseline (speedup 1.0000x reference)
"""Trainium2 Bass kernel for nn_MentionScore.

Strategy: sort spans by start, shard 2048 consecutive sorted spans per core.
Each core only touches a ~1.1k-token window of states/embeds (host passes the
window pre-transposed, bf16). The ragged gather/softmax/weighted-sum becomes
dense matmuls against one-hot / banded matrices built on-device with
iota-compare tensor ops. Layer-1 of the span MLP is algebraically folded:
  h1 = relu(OH_s.T@P1 + OH_e.T@P2 + Wg.T@P3 + onehot(len).T@WB)
with P1=states@W1a, P2=states@W1b, P3=embeds@W1c precomputed per token and
WB = width_table@W1d + b1.
"""

import sys
import types

import numpy as np
import ml_dtypes

import concourse.bass as bass
import concourse.mybir as mybir
from concourse.ap import AP
from concourse.tile import TileContext
from concourse.vector_clock import ScopedClock

BF = mybir.dt.bfloat16
F32 = mybir.dt.float32
F8 = mybir.dt.float8e4
PM = mybir.MatmulPerfMode
AT = mybir.AluOpType
AF = mybir.ActivationFunctionType
AX = mybir.AxisListType
bf16 = ml_dtypes.bfloat16
f8e4 = ml_dtypes.float8_e4m3
WSCALE = 16.0

N_CORES = 8
T, NSPAN, D, HID, LMAX, WD = 8192, 16384, 1024, 1024, 10, 20
C = NSPAN // N_CORES          # spans per core
G = C // 128                  # 128-span groups per core


class PatchedTileContext(TileContext):
    """Workaround: walrus rejects the tail Drain when it carries >1 sem wait
    ("Too many sync wait commands"). Put each wait on its own NoOp instead."""

    def _drain_and_barrier(self, tick_clock, wait_clock):
        nc = self.nc
        drain_inst = nc.sync.drain()
        wait_clock.add_sem_waits(
            drain_inst.ins, ScopedClock({None: tick_clock.global_clock})
        )
        si = drain_inst.ins.sync_info
        if si is not None and si.on_wait is not None and len(si.on_wait) > 1:
            waits = list(si.on_wait)
            drain_inst.ins.sync_info = mybir.SyncInfo(
                on_wait=[waits[0]], on_update=list(si.on_update or [])
            )
            for w in waits[1:]:
                nop = nc.sync.nop()
                nop.ins.sync_info = mybir.SyncInfo(on_wait=[w], on_update=[])

        nc.all_engine_barrier()
        assert self.sems is not None
        popped = nc._tile_sem_poison_stack.pop()
        assert popped is self._sem_poison
        nc.clear_and_free_semaphores(list(self.sems.allocated().values()))
        nc.all_engine_barrier()


def _ceil128(x):
    return int(-(-int(x) // 128) * 128)


def _plan(span_starts, span_lengths):
    """Host-side sharding plan. Returns per-core data + static layout consts."""
    order = np.argsort(span_starts, kind="stable").astype(np.int64)
    ss = span_starts[order].reshape(N_CORES, C).astype(np.int64)
    sl = span_lengths[order].reshape(N_CORES, C).astype(np.int64)
    core_base = ss[:, 0].copy()
    sloc = ss - core_base[:, None]
    eloc = sloc + sl

    T_cap = _ceil128(int(eloc.max()) + 1)
    # unaligned, shared-across-cores group window bases + per-group k-tiles
    mn = sloc[:, ::128].min(axis=0)                       # [G]
    mx = eloc.reshape(N_CORES, G, 128).max(axis=2).max(axis=0)  # [G]
    need = mx - mn + 1
    kcs = np.maximum((need + 127) // 128, 1)
    T_pad = T_cap + 128
    bases = mn.copy()
    for _ in range(3):
        bases = np.minimum(mn, T_pad - kcs * 128)
        bad = (mx - bases + 1) > kcs * 128
        if not bad.any():
            break
        kcs[bad] += 1
    K_WIN = int(kcs.max()) * 128
    d = sloc - np.repeat(bases, 128)[None, :]
    assert d.min() >= 0 and ((d + sl).reshape(N_CORES, G, 128).max(axis=2)
                             <= kcs[None, :] * 128 - 1).all(), "window overflow"

    return {
        "order": order,
        "core_base": core_base,
        "sloc": sloc,
        "d": d.astype(np.float64),
        "dl": (d + sl).astype(np.float64),
        "ln": sl.astype(np.float64),
        "T_cap": T_cap,
        "K_WIN": int(K_WIN),
        "bases": [int(b) for b in bases],
        "kcs": [int(k) for k in kcs],
    }


NGROUPS = G
SPLIT_WAITS = True


def _build(T_cap, K_WIN, bases, kcs, b3val):
    """Build the single SPMD Bass program (static; shared by all 8 cores)."""
    TC = T_cap // 128
    KC = K_WIN // 128
    T_pad = T_cap + 128
    nc = bass.Bass()

    def par(name, shape, dt):
        return nc.declare_dram_parameter(name, list(shape), dt, isOutput=False)

    NBLK = sum(kcs)
    boff = [0]
    for k in kcs:
        boff.append(boff[-1] + k)

    statesT_p = par("statesT", [D, T_cap], BF)
    embedsT_p = par("embedsT", [D, T_cap], BF)
    dmat_p = par("dmat", [128, G], F32)
    demat_p = par("demat", [128, G], F32)
    ohs_p = par("ohs", [128, NBLK * 128], BF)
    ohe_p = par("ohe", [128, NBLK * 128], BF)
    statesTf_p = par("statesTf", [D, T_cap], F8)
    aw1f_p = par("aw1f", [128, 8 * HID], F8)
    aw2f_p = par("aw2f", [128, 8 * HID], F8)
    aw3_p = par("aw3m", [128, 8], BF)
    ab1_p = par("ab1m", [128, 8], F32)
    ab2_p = par("ab2m", [128, 8], F32)
    w1a_p = par("w1a", [D, HID], BF)
    w1b_p = par("w1b", [D, HID], BF)
    w1c_p = par("w1c", [D, HID], BF)
    wbg_p = par("wbg", [128, G * HID], BF)
    w2_p = par("w2", [HID, HID], BF)
    b2_p = par("b2m", [128, 8], F32)
    w3_p = par("w3m", [128, 8], BF)
    iotaW_p = par("iotaW", [1, K_WIN], F32)
    ident_p = par("ident", [128, 128], BF)
    scores_p = nc.declare_dram_parameter("scores", [1, C], F32, isOutput=True)

    with PatchedTileContext(nc) as tc:
        with (
            tc.tile_pool(name="pp", bufs=1) as pp,
            tc.tile_pool(name="wst", bufs=2) as wst,
            tc.tile_pool(name="gp", bufs=2) as gp,
            tc.tile_pool(name="ps", bufs=2, space="PSUM") as ps,
            tc.tile_pool(name="dp", bufs=1, space="DRAM") as dp,
        ):
            dma = nc.sync.dma_start
            nblocks = [(n0, min(512, T_cap - n0))
                       for n0 in range(0, T_cap, 512)]

            def load_sTw8(n0, nw):
                t = wst.tile([128, 8, 512], F8, name="sTw8", tag="sTw8",
                             bufs=2)
                for k in range(8):
                    dma(out=t[:, k : k + 1, :nw],
                        in_=statesTf_p[k * 128 : (k + 1) * 128, n0 : n0 + nw])
                return t

            def load_tok(param, pfx, n0, nw, bufs):
                tiles = []
                for k in range(8):
                    t = wst.tile([128, 512], BF, name=f"{pfx}{k}",
                                 tag=f"{pfx}{k}", bufs=bufs)
                    dma(out=t[:, :nw],
                        in_=param[k * 128 : (k + 1) * 128, n0 : n0 + nw])
                    tiles.append(t)
                return tiles

            # critical path first: fp8 attn weights + block-0 tokens
            aw1f_t = pp.tile([128, 8, HID], F8, name="aw1f", tag="aw1f")
            dma(out=aw1f_t[:], in_=aw1f_p[:])
            sTw8_0 = load_sTw8(*nblocks[0])

            # ---------- constants / scalars ----------
            iotaW_t = pp.tile([128, K_WIN], F32, name="iotaW", tag="iotaW")
            dma(out=iotaW_t[:], in_=iotaW_p[:].partition_broadcast(128))
            ident_t = pp.tile([128, 128], BF, name="ident", tag="ident")
            dma(out=ident_t[:], in_=ident_p[:])
            dmat_t = pp.tile([128, G], F32, name="dmat", tag="dmat")
            dma(out=dmat_t[:], in_=dmat_p[:])
            demat_t = pp.tile([128, G], F32, name="demat", tag="demat")
            dma(out=demat_t[:], in_=demat_p[:])
            ab1_t = pp.tile([128, 8], F32, name="ab1", tag="ab1")
            dma(out=ab1_t[:], in_=ab1_p[:])
            ab2_t = pp.tile([128, 8], F32, name="ab2", tag="ab2")
            dma(out=ab2_t[:], in_=ab2_p[:])
            b2_t = pp.tile([128, 8], F32, name="b2", tag="b2")
            dma(out=b2_t[:], in_=b2_p[:])
            aw3_t = pp.tile([128, 8], BF, name="aw3", tag="aw3")
            dma(out=aw3_t[:], in_=aw3_p[:])
            w3_t = pp.tile([128, 8], BF, name="w3", tag="w3")
            dma(out=w3_t[:], in_=w3_p[:])

            attns_dram = dp.tile(
                [T_pad + 16], BF, name="attns_dram", tag="attns_dram")

            if NGROUPS < G:  # debug builds: ensure output is written
                zsc = pp.tile([1, C], F32, name="zsc", tag="zsc")
                nc.vector.memset(zsc[:], 0.0)
                dma(out=scores_p[:], in_=zsc[:])

            # ---------- weight slots ----------
            def wload(param, tag_prefix):
                tiles = []
                for k in range(8):
                    t = pp.tile([128, HID], BF, name=f"{tag_prefix}{k}",
                                tag=f"{tag_prefix}{k}")
                    dma(out=t[:], in_=param[k * 128 : (k + 1) * 128, :])
                    tiles.append(t)
                return tiles

            aw2f_t = pp.tile([128, 8, HID], F8, name="aw2f", tag="aw2f")
            dma(out=aw2f_t[:], in_=aw2f_p[:])
            w1a_t = wload(w1a_p, "wWA")
            sTw_0 = load_tok(statesT_p, "sTw", nblocks[0][0], nblocks[0][1], 2)
            w1b_t = wload(w1b_p, "wWB")
            w1c_t = wload(w1c_p, "wWC")
            eTw_0 = load_tok(embedsT_p, "eTw", nblocks[0][0], nblocks[0][1], 2)

            # ---------- P targets in DRAM, split at row PSPLIT so early
            # group windows stop depending on the tail of the projection ----
            PSPLIT = 640
            PdA = [dp.tile([PSPLIT, HID], BF, name=f"P{i}a", tag=f"P{i}a")
                   for i in range(3)]
            PdB = [dp.tile([T_pad - PSPLIT, HID], BF, name=f"P{i}b",
                           tag=f"P{i}b") for i in range(3)]
            zrow = pp.tile([128, HID], BF, name="zrow", tag="zrow")
            nc.vector.memset(zrow[:], 0.0)
            for pd in PdB:
                dma(out=pd[T_cap - PSPLIT :, :], in_=zrow[:])

            def p_write(pi, r0, h0, src_ap):
                # r0 is 128-aligned so a chunk never straddles PSPLIT
                if r0 < PSPLIT:
                    dma(out=PdA[pi][r0 : r0 + 128, h0 : h0 + 512], in_=src_ap)
                else:
                    dma(out=PdB[pi][r0 - PSPLIT : r0 - PSPLIT + 128,
                                    h0 : h0 + 512], in_=src_ap)

            def p_read(pi, r0, dst):
                # window rows [r0, r0+128) may straddle PSPLIT
                if r0 + 128 <= PSPLIT:
                    dma(out=dst[:], in_=PdA[pi][r0 : r0 + 128, :])
                elif r0 >= PSPLIT:
                    dma(out=dst[:], in_=PdB[pi][r0 - PSPLIT : r0 - PSPLIT + 128, :])
                else:
                    rr = PSPLIT - r0
                    dma(out=dst[:rr, :], in_=PdA[pi][r0:PSPLIT, :])
                    dma(out=dst[rr:, :], in_=PdB[pi][: 128 - rr, :])

            # ---------- blocked token pipeline: attn MLP + P projections --
            attns_t = pp.tile([1, T_cap], BF, name="attns", tag="attns")

            def emit_P(sTw, eTw, n0, nw):
                for j in range(nw // 128):
                    js = slice(j * 128, (j + 1) * 128)
                    for pi, (wt_, srcs) in enumerate(
                            ((w1a_t, sTw), (w1b_t, sTw), (w1c_t, eTw))):
                        for h0 in (0, 512):
                            pt = ps.tile([128, 512], F32, name="big",
                                         tag="big", bufs=2)
                            for k in range(8):
                                nc.tensor.matmul(
                                    pt[:], srcs[k][:, js],
                                    wt_[k][:, h0 : h0 + 512],
                                    start=(k == 0), stop=(k == 7))
                            stg = wst.tile([128, 512], BF, name=f"pstg{pi}",
                                           tag=f"pstg{pi}", bufs=2)
                            nc.scalar.copy(stg[:], pt[:])
                            p_write(pi, n0 + j * 128, h0, stg[:])

            pend_P = None
            for bi, (n0, nw) in enumerate(nblocks):
                sTw8 = sTw8_0 if bi == 0 else load_sTw8(n0, nw)
                sTw = sTw_0 if bi == 0 else load_tok(statesT_p, "sTw", n0, nw, 2)
                eTw = eTw_0 if bi == 0 else load_tok(embedsT_p, "eTw", n0, nw, 2)
                h1a8 = wst.tile([128, 8, 512], F8, name="h1a8", tag="h1a8",
                                bufs=1)
                h2a = [wst.tile([128, 512], BF, name=f"h2a{h}", tag=f"h2a{h}", bufs=1)
                       for h in range(8)]
                for hc in range(8):
                    pt = ps.tile([128, 512], F32, name="big", tag="big", bufs=2)
                    for p in range(4):
                        nc.tensor.matmul(
                            pt[:, :nw],
                            aw1f_t[:, 2 * p : 2 * p + 2,
                                   hc * 128 : (hc + 1) * 128],
                            sTw8[:, 2 * p : 2 * p + 2, :nw],
                            start=(p == 0), stop=(p == 3),
                            perf_mode=PM.DoubleRow)
                    nc.scalar.activation(
                        h1a8[:, hc : hc + 1, :nw], pt[:, :nw], AF.Relu,
                        bias=ab1_t[:, hc : hc + 1], scale=1.0 / WSCALE)
                for hc in range(8):
                    pt = ps.tile([128, 512], F32, name="big", tag="big", bufs=2)
                    for p in range(4):
                        nc.tensor.matmul(
                            pt[:, :nw],
                            aw2f_t[:, 2 * p : 2 * p + 2,
                                   hc * 128 : (hc + 1) * 128],
                            h1a8[:, 2 * p : 2 * p + 2, :nw],
                            start=(p == 0), stop=(p == 3),
                            perf_mode=PM.DoubleRow)
                    nc.scalar.activation(
                        h2a[hc][:, :nw], pt[:, :nw], AF.Relu,
                        bias=ab2_t[:, hc : hc + 1], scale=1.0 / WSCALE)
                pt = ps.tile([1, 512], F32, name="big", tag="big", bufs=2)
                for k in range(8):
                    nc.tensor.matmul(
                        pt[:, :nw], aw3_t[:, k : k + 1], h2a[k][:, :nw],
                        start=(k == 0), stop=(k == 7))
                nc.scalar.activation(attns_t[:, n0 : n0 + nw], pt[:, :nw],
                                     AF.Copy)
                dma(out=attns_dram[n0 : n0 + nw], in_=attns_t[0:1, n0 : n0 + nw])
                emit_P(sTw, eTw, n0, nw)

            zpad_t = pp.tile([1, 144], BF, name="zpad", tag="zpad")
            nc.vector.memset(zpad_t[:], 0.0)
            dma(out=attns_dram[T_cap:], in_=zpad_t[0:1, :])

            # ---------- span groups ----------
            w2_t = wload(w2_p, "wA")     # reuse again for L2
            # one-hot gather matrices, precomputed on host ([token, span])
            ohs_t = pp.tile([128, NBLK * 128], BF, name="ohs", tag="ohs")
            dma(out=ohs_t[:], in_=ohs_p[:])
            ohe_t = pp.tile([128, NBLK * 128], BF, name="ohe", tag="ohe")
            dma(out=ohe_t[:], in_=ohe_p[:])

            # Software pipeline: iteration g emits softmax+gather for group g
            # and the (relu, h1-transpose) for group g-1; the L2/L3 block for
            # groups 4b..4b+3 is emitted at the start of iteration 4b+4.
            h1b_cur = h1b_prev = None
            pend = None          # (hp, h1, gcol, h1b) awaiting relu+transpose

            def emit_relu_transpose(pend):
                hp, h1, gcol, h1b, wbg_t, S = pend
                nc.vector.tensor_tensor(out=S[:], in0=hp[:], in1=wbg_t[:],
                                        op=AT.add)
                for h0 in (0, 512):
                    nc.scalar.activation(h1[:, h0 : h0 + 512],
                                         S[:, h0 : h0 + 512], AF.Relu)
                for hc in range(8):
                    trp = ps.tile([128, 128], BF, name="tr", tag="tr", bufs=2)
                    nc.tensor.transpose(
                        trp[:], h1[:, hc * 128 : (hc + 1) * 128], ident_t[:])
                    nc.vector.tensor_copy(
                        out=h1b[hc][:, gcol : gcol + 128], in_=trp[:])

            def emit_l2_l3(h1b, b0, c0=0, cw=512):
                cs = slice(c0, c0 + cw)
                h2b = [gp.tile([128, 512], BF, name=f"h2b{k}", tag=f"h2b{k}",
                               bufs=1) for k in range(8)]
                for h2c in range(8):
                    pt = ps.tile([128, 512], F32, name="big", tag="big", bufs=2)
                    for k in range(8):
                        nc.tensor.matmul(
                            pt[:, :cw], w2_t[k][:, h2c * 128 : (h2c + 1) * 128],
                            h1b[k][:, cs], start=(k == 0), stop=(k == 7))
                    nc.scalar.activation(h2b[h2c][:, :cw], pt[:, :cw], AF.Relu,
                                         bias=b2_t[:, h2c : h2c + 1])
                pt = ps.tile([1, 512], F32, name="big", tag="big", bufs=2)
                for k in range(8):
                    nc.tensor.matmul(pt[:, :cw], w3_t[:, k : k + 1],
                                     h2b[k][:, :cw],
                                     start=(k == 0), stop=(k == 7))
                ob = gp.tile([1, 512], F32, name="ob", tag="ob")
                nc.scalar.activation(ob[:, :cw], pt[:, :cw], AF.Copy,
                                     bias=float(b3val))
                dma(out=scores_p[:, b0 + c0 : b0 + c0 + cw], in_=ob[:, :cw])

            for g in range(NGROUPS):
                KCg = kcs[g]
                WW = KCg * 128
                if g % 4 == 0:
                    h1b_prev = h1b_cur
                    h1b_cur = [gp.tile([128, 512], BF, name=f"h1b{k}",
                                       tag=f"h1b{k}", bufs=2) for k in range(8)]
                gcol = (g % 4) * 128

                # P windows from DRAM; reuse the idle w1a/b/c weight slots as
                # a manual double buffer (alternating on g%2).
                pw = []
                for pi, pfx in enumerate(("wWA", "wWB", "wWC")):
                    tiles = []
                    for kk in range(KCg):
                        pt_ = pp.tile([128, HID], BF,
                                      name=f"{pfx}{kk * 4 + g % 4}",
                                      tag=f"{pfx}{kk * 4 + g % 4}")
                        p_read(pi, bases[g] + kk * 128, pt_)
                        tiles.append(pt_)
                    pw.append(tiles)

                wbg_t = gp.tile([128, HID], BF, name="wbg", tag="wbg",
                                bufs=2)
                dma(out=wbg_t[:], in_=wbg_p[:, g * HID : (g + 1) * HID])
                # attns window, broadcast to all partitions: A[n, t]
                A_t = gp.tile([128, K_WIN], BF, name="A", tag="A", bufs=2)
                dma(out=A_t[:, :WW],
                    in_=attns_dram[bases[g] : bases[g] + WW]
                    .partition_broadcast(128))
                # exp early on scalar queue (before prev group's relu)
                EA = gp.tile([128, K_WIN], F32, name="EA", tag="EA", bufs=2)
                nc.scalar.activation(EA[:, :WW], A_t[:, :WW], AF.Exp)

                # band mask d[n] <= t <= d[n]+len[n], softmax over the band
                m1 = gp.tile([128, K_WIN], F32, name="m1", tag="m1", bufs=1)
                nc.vector.tensor_scalar(out=m1[:, :WW], in0=iotaW_t[:, :WW],
                                        scalar1=dmat_t[:, g : g + 1],
                                        scalar2=None, op0=AT.is_ge)
                m2 = gp.tile([128, K_WIN], F32, name="m2", tag="m2", bufs=1)
                nc.vector.tensor_scalar(out=m2[:, :WW], in0=iotaW_t[:, :WW],
                                        scalar1=demat_t[:, g : g + 1],
                                        scalar2=None, op0=AT.is_le)
                band = gp.tile([128, K_WIN], F32, name="band", tag="band", bufs=1)
                nc.vector.tensor_tensor(out=band[:, :WW], in0=m1[:, :WW],
                                        in1=m2[:, :WW], op=AT.mult)
                EW = gp.tile([128, K_WIN], F32, name="EW", tag="EW", bufs=1)
                nc.vector.tensor_tensor(out=EW[:, :WW], in0=EA[:, :WW],
                                        in1=band[:, :WW], op=AT.mult)
                rsum = gp.tile([128, 1], F32, name="rsum", tag="rsum", bufs=1)
                nc.vector.tensor_reduce(out=rsum[:], in_=EW[:, :WW],
                                        axis=AX.X, op=AT.add)
                rinv = gp.tile([128, 1], F32, name="rinv", tag="rinv", bufs=1)
                nc.vector.reciprocal(rinv[:], rsum[:])
                wg = gp.tile([128, K_WIN], BF, name="wg", tag="wg", bufs=2)
                nc.vector.tensor_scalar(out=wg[:, :WW], in0=EW[:, :WW],
                                        scalar1=rinv[:, 0:1], scalar2=None,
                                        op0=AT.mult)

                # PE-transpose wg → wgT [token, span]
                wgT = gp.tile([128, K_WIN], BF, name="wgT", tag="wgT", bufs=2)
                for kk in range(KCg):
                    trp = ps.tile([128, 128], BF, name="tr", tag="tr", bufs=2)
                    nc.tensor.transpose(
                        trp[:], wg[:, kk * 128 : (kk + 1) * 128], ident_t[:])
                    nc.scalar.copy(wgT[:, kk * 128 : (kk + 1) * 128], trp[:])

                # flipped gather: h1[span, hid] += OH.T @ Pwin, N=512
                hp = ps.tile([128, HID], F32, name="hp", tag="hp", bufs=2)
                steps = []
                for kk in range(KCg):
                    bs = slice((boff[g] + kk) * 128, (boff[g] + kk + 1) * 128)
                    steps.append((ohs_t[:, bs], pw[0][kk]))
                    steps.append((ohe_t[:, bs], pw[1][kk]))
                for kk in range(KCg):
                    steps.append((wgT[:, kk * 128 : (kk + 1) * 128], pw[2][kk]))
                for h0 in (0, 512):
                    for i, (lhsT, rhs) in enumerate(steps):
                        nc.tensor.matmul(hp[:, h0 : h0 + 512], lhsT,
                                         rhs[:, h0 : h0 + 512],
                                         start=(i == 0),
                                         stop=(i == len(steps) - 1))

                if pend is not None:
                    emit_relu_transpose(pend)
                if g % 4 == 0 and g > 0:
                    emit_l2_l3(h1b_prev, (g // 4 - 1) * 512)
                if g == NGROUPS - 2:
                    # first half of the final block (groups 12-13) early, so
                    # only a half-width L2/L3 remains after the last gather
                    emit_l2_l3(h1b_cur, (NGROUPS // 4 - 1) * 512, 0, 256)
                h1 = gp.tile([128, HID], BF, name="h1", tag="h1", bufs=2)
                S = gp.tile([128, HID], F32, name="S", tag="S", bufs=2)
                pend = (hp, h1, gcol, h1b_cur, wbg_t, S)

            emit_relu_transpose(pend)
            emit_l2_l3(h1b_cur, (NGROUPS // 4 - 1) * 512, 256, 256)

    if SPLIT_WAITS:
        _split_waits(nc)
    return nc


def _split_waits(nc, max_waits=1):
    """This walrus build rejects instructions carrying >max_waits sem waits
    ("Too many sync wait commands"). Hoist excess waits onto same-engine
    NoOps placed immediately before the instruction — identical semantics
    (engine queues are in-order)."""
    ctr = [0]
    for f in nc.m.functions:
        for blk in f.blocks:
            out = []
            for ins in blk.instructions:
                si = getattr(ins, "sync_info", None)
                if si is not None and si.on_wait and len(si.on_wait) > max_waits:
                    waits = list(si.on_wait)
                    for w in waits[:-max_waits]:
                        ctr[0] += 1
                        nop = mybir.InstNoOp(
                            name=f"I-wsplit-{ctr[0]}", ins=[], outs=[],
                            sync_info=mybir.SyncInfo(on_wait=[w], on_update=[]),
                        )
                        nop.engine = ins.engine
                        out.append(nop)
                    ins.sync_info = mybir.SyncInfo(
                        on_wait=waits[-max_waits:],
                        on_update=list(si.on_update or []),
                    )
                out.append(ins)
            blk.instructions[:] = out
    return ctr[0]


_CACHE = {}
LAST_EXEC_NS = None
TRACE = False


def _install_ntff_shim():
    try:
        import antenv.axon_hooks  # noqa: F401
        return
    except ImportError:
        pass
    try:
        from trn_agent_boot.trn_boot import _ntff_profile_via_ctypes
        hook = _ntff_profile_via_ctypes("/opt/axon/libaxon_pjrt.so")
    except Exception:
        hook = None
    m1 = types.ModuleType("antenv")
    m2 = types.ModuleType("antenv.axon_hooks")
    m2.get_axon_ntff_profile_hook = lambda: hook
    m2.set_axon_ntff_profile_hook = lambda h: None
    m1.axon_hooks = m2
    sys.modules.setdefault("antenv", m1)
    sys.modules["antenv.axon_hooks"] = m2


def _prepare(inputs):
    inp = {k: np.asarray(v) for k, v in inputs.items()}
    ss = inp["span_starts"].astype(np.int64)
    sl = inp["span_lengths"].astype(np.int64)
    plan = _plan(ss, sl)
    T_cap, K_WIN, bases = plan["T_cap"], plan["K_WIN"], plan["bases"]
    KC = K_WIN // 128
    b3val = float(np.asarray(inp["score_b3"]).reshape(-1)[0])

    kcs = plan["kcs"]
    key = (T_cap, K_WIN, tuple(bases), tuple(kcs), b3val)
    if key not in _CACHE:
        _CACHE[key] = _build(T_cap, K_WIN, bases, kcs, b3val)
    nc = _CACHE[key]

    def bfc(x):
        return np.ascontiguousarray(np.asarray(x, dtype=np.float32)).astype(bf16)

    sw1 = inp["score_w1"].astype(np.float32)

    def f8pack(w):
        # [1024,1024] -> [128, ktile, m] fp8, pre-scaled by WSCALE
        a = (np.asarray(w, np.float32) * WSCALE).reshape(8, 128, HID)
        return np.ascontiguousarray(
            a.transpose(1, 0, 2).reshape(128, 8 * HID)).astype(f8e4)

    shared = {
        "aw1f": f8pack(inp["attn_w1"]),
        "aw2f": f8pack(inp["attn_w2"]),
        "aw3m": bfc(inp["attn_w3"].reshape(8, 128).T),
        "ab1m": np.ascontiguousarray(
            inp["attn_b1"].astype(np.float32).reshape(8, 128).T),
        "ab2m": np.ascontiguousarray(
            inp["attn_b2"].astype(np.float32).reshape(8, 128).T),
        "w1a": bfc(sw1[0:1024]),
        "w1b": bfc(sw1[1024:2048]),
        "w1c": bfc(sw1[2048:3072]),
        "w2": bfc(inp["score_w2"]),
        "b2m": np.ascontiguousarray(
            inp["score_b2"].astype(np.float32).reshape(8, 128).T),
        "w3m": bfc(inp["score_w3"].reshape(8, 128).T),
        "iotaW": np.arange(K_WIN, dtype=np.float32).reshape(1, -1),
        "ident": np.eye(128, dtype=np.float32).astype(bf16),
    }

    states = inp["states"].astype(np.float32)
    embeds = inp["embeds"].astype(np.float32)
    in_maps = []
    for c in range(N_CORES):
        cb = int(plan["core_base"][c])
        stl = np.zeros((T_cap, D), np.float32)
        eml = np.zeros((T_cap, D), np.float32)
        hi = min(T, cb + T_cap)
        stl[: hi - cb] = states[cb:hi]
        eml[: hi - cb] = embeds[cb:hi]
        m = dict(shared)
        m["statesT"] = np.ascontiguousarray(stl.T).astype(bf16)
        m["statesTf"] = np.ascontiguousarray(stl.T).astype(f8e4)
        m["embedsT"] = np.ascontiguousarray(eml.T).astype(bf16)
        d = plan["d"][c].astype(np.float32)
        dl = plan["dl"][c].astype(np.float32)
        ln = plan["ln"][c].astype(np.int64)
        m["dmat"] = np.ascontiguousarray(d.reshape(G, 128).T)
        m["demat"] = np.ascontiguousarray(dl.reshape(G, 128).T)
        # host-built one-hot gather matrices, [token, span] layout
        NBLK = sum(kcs)
        boff = np.cumsum([0] + kcs)
        di = plan["d"][c].astype(np.int64).reshape(G, 128)
        dei = plan["dl"][c].astype(np.int64).reshape(G, 128)
        ohs = np.zeros((128, NBLK * 128), np.float32)
        ohe = np.zeros((128, NBLK * 128), np.float32)
        cols = np.arange(128)
        for g in range(G):
            for kk in range(kcs[g]):
                c0 = (boff[g] + kk) * 128
                for arr, idx in ((ohs, di[g]), (ohe, dei[g])):
                    r = idx - kk * 128
                    sel = (r >= 0) & (r < 128)
                    arr[r[sel], c0 + cols[sel]] = 1.0
        m["ohs"] = ohs.astype(bf16)
        m["ohe"] = ohe.astype(bf16)
        WB32 = (inp["width_table"].astype(np.float64)
                @ sw1[3072:3092].astype(np.float64)
                + inp["score_b1"].astype(np.float64)).astype(np.float32)
        m["wbg"] = np.ascontiguousarray(
            WB32[ln].reshape(G, 128, HID).transpose(1, 0, 2)
            .reshape(128, G * HID)).astype(bf16)
        in_maps.append(m)

    return nc, in_maps, plan


def kernel(**inputs):
    global LAST_EXEC_NS
    from concourse.bass_utils import run_bass_kernel_spmd

    nc, in_maps, plan = _prepare(inputs)
    _install_ntff_shim()
    res = run_bass_kernel_spmd(nc, in_maps, list(range(N_CORES)), trace=TRACE)
    LAST_EXEC_NS = res.exec_time_ns

    out = np.empty(NSPAN, np.float32)
    for c in range(N_CORES):
        out[plan["order"][c * C : (c + 1) * C]] = np.asarray(
            res.results[c]["scores"]).reshape(-1)
    return out.reshape(NSPAN, 1)



# revision 30
# speedup vs baseline: 1.0441x; 1.0441x over previous
"""Trainium2 Bass kernel for nn_MentionScore.

Strategy: sort spans by start, shard 2048 consecutive sorted spans per core.
Each core only touches a ~1.1k-token window of states/embeds (host passes the
window pre-transposed, bf16). The ragged gather/softmax/weighted-sum becomes
dense matmuls against one-hot / banded matrices built on-device with
iota-compare tensor ops. Layer-1 of the span MLP is algebraically folded:
  h1 = relu(OH_s.T@P1 + OH_e.T@P2 + Wg.T@P3 + onehot(len).T@WB)
with P1=states@W1a, P2=states@W1b, P3=embeds@W1c precomputed per token and
WB = width_table@W1d + b1.
"""

import sys
import types

import numpy as np
import ml_dtypes

import concourse.bass as bass
import concourse.mybir as mybir
from concourse.ap import AP
from concourse.tile import TileContext
from concourse.vector_clock import ScopedClock

BF = mybir.dt.bfloat16
F32 = mybir.dt.float32
F8 = mybir.dt.float8e4
PM = mybir.MatmulPerfMode
AT = mybir.AluOpType
AF = mybir.ActivationFunctionType
AX = mybir.AxisListType
bf16 = ml_dtypes.bfloat16
f8e4 = ml_dtypes.float8_e4m3
WSCALE = 16.0

N_CORES = 8
T, NSPAN, D, HID, LMAX, WD = 8192, 16384, 1024, 1024, 10, 20
C = NSPAN // N_CORES          # spans per core
G = C // 128                  # 128-span groups per core


class PatchedTileContext(TileContext):
    """Workaround: walrus rejects the tail Drain when it carries >1 sem wait
    ("Too many sync wait commands"). Put each wait on its own NoOp instead."""

    def _drain_and_barrier(self, tick_clock, wait_clock):
        nc = self.nc
        drain_inst = nc.sync.drain()
        wait_clock.add_sem_waits(
            drain_inst.ins, ScopedClock({None: tick_clock.global_clock})
        )
        si = drain_inst.ins.sync_info
        if si is not None and si.on_wait is not None and len(si.on_wait) > 1:
            waits = list(si.on_wait)
            drain_inst.ins.sync_info = mybir.SyncInfo(
                on_wait=[waits[0]], on_update=list(si.on_update or [])
            )
            for w in waits[1:]:
                nop = nc.sync.nop()
                nop.ins.sync_info = mybir.SyncInfo(on_wait=[w], on_update=[])

        nc.all_engine_barrier()
        assert self.sems is not None
        popped = nc._tile_sem_poison_stack.pop()
        assert popped is self._sem_poison
        nc.clear_and_free_semaphores(list(self.sems.allocated().values()))
        nc.all_engine_barrier()


def _ceil128(x):
    return int(-(-int(x) // 128) * 128)


def _plan(span_starts, span_lengths):
    """Host-side sharding plan. Returns per-core data + static layout consts."""
    order = np.argsort(span_starts, kind="stable").astype(np.int64)
    ss = span_starts[order].reshape(N_CORES, C).astype(np.int64)
    sl = span_lengths[order].reshape(N_CORES, C).astype(np.int64)
    core_base = ss[:, 0].copy()
    sloc = ss - core_base[:, None]
    eloc = sloc + sl

    T_cap = _ceil128(int(eloc.max()) + 1)
    # unaligned, shared-across-cores group window bases + per-group k-tiles
    mn = sloc[:, ::128].min(axis=0)                       # [G]
    mx = eloc.reshape(N_CORES, G, 128).max(axis=2).max(axis=0)  # [G]
    need = mx - mn + 1
    kcs = np.maximum((need + 127) // 128, 1)
    T_pad = T_cap + 128
    bases = mn.copy()
    for _ in range(3):
        bases = np.minimum(mn, T_pad - kcs * 128)
        bad = (mx - bases + 1) > kcs * 128
        if not bad.any():
            break
        kcs[bad] += 1
    K_WIN = int(kcs.max()) * 128
    d = sloc - np.repeat(bases, 128)[None, :]
    assert d.min() >= 0 and ((d + sl).reshape(N_CORES, G, 128).max(axis=2)
                             <= kcs[None, :] * 128 - 1).all(), "window overflow"

    return {
        "order": order,
        "core_base": core_base,
        "sloc": sloc,
        "d": d.astype(np.float64),
        "dl": (d + sl).astype(np.float64),
        "ln": sl.astype(np.float64),
        "T_cap": T_cap,
        "K_WIN": int(K_WIN),
        "bases": [int(b) for b in bases],
        "kcs": [int(k) for k in kcs],
    }


NGROUPS = G
SPLIT_WAITS = True


def _build(T_cap, K_WIN, bases, kcs, b3val):
    """Build the single SPMD Bass program (static; shared by all 8 cores)."""
    TC = T_cap // 128
    KC = K_WIN // 128
    T_pad = T_cap + 128
    nc = bass.Bass()

    def par(name, shape, dt):
        return nc.declare_dram_parameter(name, list(shape), dt, isOutput=False)

    NBLK = sum(kcs)
    boff = [0]
    for k in kcs:
        boff.append(boff[-1] + k)

    statesT_p = par("statesT", [D, T_cap], BF)
    embedsT_p = par("embedsT", [D, T_cap], BF)
    dmat_p = par("dmat", [128, G], F32)
    demat_p = par("demat", [128, G], F32)
    ohs_p = par("ohs", [128, NBLK * 128], BF)
    ohe_p = par("ohe", [128, NBLK * 128], BF)
    statesTf_p = par("statesTf", [D, T_cap], F8)
    aw1f_p = par("aw1f", [128, 8 * HID], F8)
    aw2f_p = par("aw2f", [128, 8 * HID], F8)
    aw3_p = par("aw3m", [128, 8], BF)
    ab1_p = par("ab1m", [128, 8], F32)
    ab2_p = par("ab2m", [128, 8], F32)
    w1a_p = par("w1a", [D, HID], BF)
    w1b_p = par("w1b", [D, HID], BF)
    w1c_p = par("w1c", [D, HID], BF)
    w1d_p = par("w1d", [WD, HID], BF)
    wtT_p = par("wtT", [WD, LMAX], BF)
    b1r_p = par("b1r", [1, HID], BF)
    ohl_p = par("ohl", [128, C], BF)
    w2_p = par("w2", [HID, HID], BF)
    b2_p = par("b2m", [128, 8], F32)
    w3_p = par("w3m", [128, 8], BF)
    iotaW_p = par("iotaW", [1, K_WIN], F32)
    ident_p = par("ident", [128, 128], BF)
    scores_p = nc.declare_dram_parameter("scores", [1, C], F32, isOutput=True)

    with PatchedTileContext(nc) as tc:
        with (
            tc.tile_pool(name="pp", bufs=1) as pp,
            tc.tile_pool(name="wst", bufs=2) as wst,
            tc.tile_pool(name="gp", bufs=2) as gp,
            tc.tile_pool(name="ps", bufs=2, space="PSUM") as ps,
            tc.tile_pool(name="dp", bufs=1, space="DRAM") as dp,
        ):
            dma = nc.sync.dma_start
            nblocks = [(n0, min(512, T_cap - n0))
                       for n0 in range(0, T_cap, 512)]

            def load_sTw8(n0, nw):
                t = wst.tile([128, 8, 512], F8, name="sTw8", tag="sTw8",
                             bufs=2)
                for k in range(8):
                    dma(out=t[:, k : k + 1, :nw],
                        in_=statesTf_p[k * 128 : (k + 1) * 128, n0 : n0 + nw])
                return t

            def load_tok(param, pfx, n0, nw, bufs):
                tiles = []
                for k in range(8):
                    t = wst.tile([128, 512], BF, name=f"{pfx}{k}",
                                 tag=f"{pfx}{k}", bufs=bufs)
                    dma(out=t[:, :nw],
                        in_=param[k * 128 : (k + 1) * 128, n0 : n0 + nw])
                    tiles.append(t)
                return tiles

            # critical path first: fp8 attn weights + block-0 tokens
            aw1f_t = pp.tile([128, 8, HID], F8, name="aw1f", tag="aw1f")
            dma(out=aw1f_t[:], in_=aw1f_p[:])
            sTw8_0 = load_sTw8(*nblocks[0])

            # ---------- constants / scalars ----------
            iotaW_t = pp.tile([128, K_WIN], F32, name="iotaW", tag="iotaW")
            dma(out=iotaW_t[:], in_=iotaW_p[:].partition_broadcast(128))
            ident_t = pp.tile([128, 128], BF, name="ident", tag="ident")
            dma(out=ident_t[:], in_=ident_p[:])
            ones16_t = pp.tile([1, 16], BF, name="ones16", tag="ones16")
            nc.vector.memset(ones16_t[:], 1.0)
            b1r_t = pp.tile([1, HID], BF, name="b1r", tag="b1r")
            dma(out=b1r_t[:], in_=b1r_p[:])
            w1d_t = pp.tile([WD, HID], BF, name="w1d", tag="w1d")
            dma(out=w1d_t[:], in_=w1d_p[:])
            wtT_t = pp.tile([WD, 16], BF, name="wtT", tag="wtT")
            nc.vector.memset(wtT_t[:], 0.0)
            dma(out=wtT_t[:, :LMAX], in_=wtT_p[:])
            dmat_t = pp.tile([128, G], F32, name="dmat", tag="dmat")
            dma(out=dmat_t[:], in_=dmat_p[:])
            demat_t = pp.tile([128, G], F32, name="demat", tag="demat")
            dma(out=demat_t[:], in_=demat_p[:])
            ab1_t = pp.tile([128, 8], F32, name="ab1", tag="ab1")
            dma(out=ab1_t[:], in_=ab1_p[:])
            ab2_t = pp.tile([128, 8], F32, name="ab2", tag="ab2")
            dma(out=ab2_t[:], in_=ab2_p[:])
            b2_t = pp.tile([128, 8], F32, name="b2", tag="b2")
            dma(out=b2_t[:], in_=b2_p[:])
            aw3_t = pp.tile([128, 8], BF, name="aw3", tag="aw3")
            dma(out=aw3_t[:], in_=aw3_p[:])
            w3_t = pp.tile([128, 8], BF, name="w3", tag="w3")
            dma(out=w3_t[:], in_=w3_p[:])

            attns_dram = dp.tile(
                [T_pad + 16], BF, name="attns_dram", tag="attns_dram")

            if NGROUPS < G:  # debug builds: ensure output is written
                zsc = pp.tile([1, C], F32, name="zsc", tag="zsc")
                nc.vector.memset(zsc[:], 0.0)
                dma(out=scores_p[:], in_=zsc[:])

            # ---------- weight slots ----------
            def wload(param, tag_prefix):
                tiles = []
                for k in range(8):
                    t = pp.tile([128, HID], BF, name=f"{tag_prefix}{k}",
                                tag=f"{tag_prefix}{k}")
                    dma(out=t[:], in_=param[k * 128 : (k + 1) * 128, :])
                    tiles.append(t)
                return tiles

            aw2f_t = pp.tile([128, 8, HID], F8, name="aw2f", tag="aw2f")
            dma(out=aw2f_t[:], in_=aw2f_p[:])
            w1a_t = wload(w1a_p, "wWA")
            sTw_0 = load_tok(statesT_p, "sTw", nblocks[0][0], nblocks[0][1], 2)
            w1b_t = wload(w1b_p, "wWB")
            w1c_t = wload(w1c_p, "wWC")
            eTw_0 = load_tok(embedsT_p, "eTw", nblocks[0][0], nblocks[0][1], 2)

            # ---------- P targets in DRAM, split at row PSPLIT so early
            # group windows stop depending on the tail of the projection ----
            PSPLIT = 640
            PdA = [dp.tile([PSPLIT, HID], BF, name=f"P{i}a", tag=f"P{i}a")
                   for i in range(3)]
            PdB = [dp.tile([T_pad - PSPLIT, HID], BF, name=f"P{i}b",
                           tag=f"P{i}b") for i in range(3)]
            zrow = pp.tile([128, HID], BF, name="zrow", tag="zrow")
            nc.vector.memset(zrow[:], 0.0)
            for pd in PdB:
                dma(out=pd[T_cap - PSPLIT :, :], in_=zrow[:])

            def p_write(pi, r0, h0, src_ap):
                # r0 is 128-aligned so a chunk never straddles PSPLIT
                if r0 < PSPLIT:
                    dma(out=PdA[pi][r0 : r0 + 128, h0 : h0 + 512], in_=src_ap)
                else:
                    dma(out=PdB[pi][r0 - PSPLIT : r0 - PSPLIT + 128,
                                    h0 : h0 + 512], in_=src_ap)

            def p_read(pi, r0, dst):
                # window rows [r0, r0+128) may straddle PSPLIT
                if r0 + 128 <= PSPLIT:
                    dma(out=dst[:], in_=PdA[pi][r0 : r0 + 128, :])
                elif r0 >= PSPLIT:
                    dma(out=dst[:], in_=PdB[pi][r0 - PSPLIT : r0 - PSPLIT + 128, :])
                else:
                    rr = PSPLIT - r0
                    dma(out=dst[:rr, :], in_=PdA[pi][r0:PSPLIT, :])
                    dma(out=dst[rr:, :], in_=PdB[pi][: 128 - rr, :])

            # ---------- blocked token pipeline: attn MLP + P projections --
            attns_t = pp.tile([1, T_cap], BF, name="attns", tag="attns")

            def emit_P(sTw, eTw, n0, nw):
                for j in range(nw // 128):
                    js = slice(j * 128, (j + 1) * 128)
                    for pi, (wt_, srcs) in enumerate(
                            ((w1a_t, sTw), (w1b_t, sTw), (w1c_t, eTw))):
                        for h0 in (0, 512):
                            pt = ps.tile([128, 512], F32, name="big",
                                         tag="big", bufs=2)
                            for k in range(8):
                                nc.tensor.matmul(
                                    pt[:], srcs[k][:, js],
                                    wt_[k][:, h0 : h0 + 512],
                                    start=(k == 0), stop=(k == 7))
                            stg = wst.tile([128, 512], BF, name=f"pstg{pi}",
                                           tag=f"pstg{pi}", bufs=2)
                            nc.scalar.copy(stg[:], pt[:])
                            p_write(pi, n0 + j * 128, h0, stg[:])

            pend_P = None
            for bi, (n0, nw) in enumerate(nblocks):
                sTw8 = sTw8_0 if bi == 0 else load_sTw8(n0, nw)
                sTw = sTw_0 if bi == 0 else load_tok(statesT_p, "sTw", n0, nw, 2)
                eTw = eTw_0 if bi == 0 else load_tok(embedsT_p, "eTw", n0, nw, 2)
                h1a8 = wst.tile([128, 8, 512], F8, name="h1a8", tag="h1a8",
                                bufs=1)
                h2a = [wst.tile([128, 512], BF, name=f"h2a{h}", tag=f"h2a{h}", bufs=1)
                       for h in range(8)]
                for hc in range(8):
                    pt = ps.tile([128, 512], F32, name="big", tag="big", bufs=2)
                    for p in range(4):
                        nc.tensor.matmul(
                            pt[:, :nw],
                            aw1f_t[:, 2 * p : 2 * p + 2,
                                   hc * 128 : (hc + 1) * 128],
                            sTw8[:, 2 * p : 2 * p + 2, :nw],
                            start=(p == 0), stop=(p == 3),
                            perf_mode=PM.DoubleRow)
                    nc.scalar.activation(
                        h1a8[:, hc : hc + 1, :nw], pt[:, :nw], AF.Relu,
                        bias=ab1_t[:, hc : hc + 1], scale=1.0 / WSCALE)
                for hc in range(8):
                    pt = ps.tile([128, 512], F32, name="big", tag="big", bufs=2)
                    for p in range(4):
                        nc.tensor.matmul(
                            pt[:, :nw],
                            aw2f_t[:, 2 * p : 2 * p + 2,
                                   hc * 128 : (hc + 1) * 128],
                            h1a8[:, 2 * p : 2 * p + 2, :nw],
                            start=(p == 0), stop=(p == 3),
                            perf_mode=PM.DoubleRow)
                    nc.scalar.activation(
                        h2a[hc][:, :nw], pt[:, :nw], AF.Relu,
                        bias=ab2_t[:, hc : hc + 1], scale=1.0 / WSCALE)
                pt = ps.tile([1, 512], F32, name="big", tag="big", bufs=2)
                for k in range(8):
                    nc.tensor.matmul(
                        pt[:, :nw], aw3_t[:, k : k + 1], h2a[k][:, :nw],
                        start=(k == 0), stop=(k == 7))
                nc.scalar.activation(attns_t[:, n0 : n0 + nw], pt[:, :nw],
                                     AF.Copy)
                dma(out=attns_dram[n0 : n0 + nw], in_=attns_t[0:1, n0 : n0 + nw])
                emit_P(sTw, eTw, n0, nw)

            zpad_t = pp.tile([1, 144], BF, name="zpad", tag="zpad")
            nc.vector.memset(zpad_t[:], 0.0)
            dma(out=attns_dram[T_cap:], in_=zpad_t[0:1, :])

            # ---- WB = width_table @ W1d + b1 → [128, HID] bf16 (zero-pad)
            WB_t = pp.tile([128, HID], BF, name="WB", tag="WB")
            nc.vector.memset(WB_t[:], 0.0)
            for n0 in range(0, HID, 512):
                pt = ps.tile([16, 512], F32, name="big", tag="big", bufs=2)
                nc.tensor.matmul(pt[:], wtT_t[:], w1d_t[:, n0 : n0 + 512],
                                 start=True, stop=False)
                nc.tensor.matmul(pt[:], ones16_t[:], b1r_t[:, n0 : n0 + 512],
                                 start=False, stop=True)
                nc.vector.tensor_copy(out=WB_t[:16, n0 : n0 + 512], in_=pt[:])

            # ---------- span groups ----------
            w2_t = wload(w2_p, "wA")     # reuse again for L2
            ohl_t = pp.tile([128, C], BF, name="ohl", tag="ohl")
            dma(out=ohl_t[:], in_=ohl_p[:])
            # one-hot gather matrices, precomputed on host ([token, span])
            ohs_t = pp.tile([128, NBLK * 128], BF, name="ohs", tag="ohs")
            dma(out=ohs_t[:], in_=ohs_p[:])
            ohe_t = pp.tile([128, NBLK * 128], BF, name="ohe", tag="ohe")
            dma(out=ohe_t[:], in_=ohe_p[:])

            # Software pipeline: iteration g emits softmax+gather for group g
            # and the (relu, h1-transpose) for group g-1; the L2/L3 block for
            # groups 4b..4b+3 is emitted at the start of iteration 4b+4.
            h1b_cur = h1b_prev = None
            pend = None          # (hp, h1, gcol, h1b) awaiting relu+transpose

            def emit_relu_transpose(pend):
                hp, h1, gcol, h1b = pend
                for h0 in (0, 512):
                    nc.scalar.activation(h1[:, h0 : h0 + 512],
                                         hp[:, h0 : h0 + 512], AF.Relu)
                for hc in range(8):
                    trp = ps.tile([128, 128], BF, name="tr", tag="tr", bufs=2)
                    nc.tensor.transpose(
                        trp[:], h1[:, hc * 128 : (hc + 1) * 128], ident_t[:])
                    nc.vector.tensor_copy(
                        out=h1b[hc][:, gcol : gcol + 128], in_=trp[:])

            def emit_l2_l3(h1b, b0, c0=0, cw=512):
                cs = slice(c0, c0 + cw)
                h2b = [gp.tile([128, 512], BF, name=f"h2b{k}", tag=f"h2b{k}",
                               bufs=1) for k in range(8)]
                for h2c in range(8):
                    pt = ps.tile([128, 512], F32, name="big", tag="big", bufs=2)
                    for k in range(8):
                        nc.tensor.matmul(
                            pt[:, :cw], w2_t[k][:, h2c * 128 : (h2c + 1) * 128],
                            h1b[k][:, cs], start=(k == 0), stop=(k == 7))
                    nc.scalar.activation(h2b[h2c][:, :cw], pt[:, :cw], AF.Relu,
                                         bias=b2_t[:, h2c : h2c + 1])
                pt = ps.tile([1, 512], F32, name="big", tag="big", bufs=2)
                for k in range(8):
                    nc.tensor.matmul(pt[:, :cw], w3_t[:, k : k + 1],
                                     h2b[k][:, :cw],
                                     start=(k == 0), stop=(k == 7))
                ob = gp.tile([1, 512], F32, name="ob", tag="ob")
                nc.scalar.activation(ob[:, :cw], pt[:, :cw], AF.Copy,
                                     bias=float(b3val))
                dma(out=scores_p[:, b0 + c0 : b0 + c0 + cw], in_=ob[:, :cw])

            for g in range(NGROUPS):
                KCg = kcs[g]
                WW = KCg * 128
                if g % 4 == 0:
                    h1b_prev = h1b_cur
                    h1b_cur = [gp.tile([128, 512], BF, name=f"h1b{k}",
                                       tag=f"h1b{k}", bufs=2) for k in range(8)]
                gcol = (g % 4) * 128

                # P windows from DRAM; reuse the idle w1a/b/c weight slots as
                # a manual double buffer (alternating on g%2).
                pw = []
                for pi, pfx in enumerate(("wWA", "wWB", "wWC")):
                    tiles = []
                    for kk in range(KCg):
                        pt_ = pp.tile([128, HID], BF,
                                      name=f"{pfx}{kk * 4 + g % 4}",
                                      tag=f"{pfx}{kk * 4 + g % 4}")
                        p_read(pi, bases[g] + kk * 128, pt_)
                        tiles.append(pt_)
                    pw.append(tiles)

                # attns window, broadcast to all partitions: A[n, t]
                A_t = gp.tile([128, K_WIN], BF, name="A", tag="A", bufs=2)
                dma(out=A_t[:, :WW],
                    in_=attns_dram[bases[g] : bases[g] + WW]
                    .partition_broadcast(128))
                # exp early on scalar queue (before prev group's relu)
                EA = gp.tile([128, K_WIN], F32, name="EA", tag="EA", bufs=2)
                nc.scalar.activation(EA[:, :WW], A_t[:, :WW], AF.Exp)

                # band mask d[n] <= t <= d[n]+len[n], softmax over the band
                m1 = gp.tile([128, K_WIN], F32, name="m1", tag="m1", bufs=1)
                nc.vector.tensor_scalar(out=m1[:, :WW], in0=iotaW_t[:, :WW],
                                        scalar1=dmat_t[:, g : g + 1],
                                        scalar2=None, op0=AT.is_ge)
                m2 = gp.tile([128, K_WIN], F32, name="m2", tag="m2", bufs=1)
                nc.vector.tensor_scalar(out=m2[:, :WW], in0=iotaW_t[:, :WW],
                                        scalar1=demat_t[:, g : g + 1],
                                        scalar2=None, op0=AT.is_le)
                band = gp.tile([128, K_WIN], F32, name="band", tag="band", bufs=1)
                nc.vector.tensor_tensor(out=band[:, :WW], in0=m1[:, :WW],
                                        in1=m2[:, :WW], op=AT.mult)
                EW = gp.tile([128, K_WIN], F32, name="EW", tag="EW", bufs=1)
                nc.vector.tensor_tensor(out=EW[:, :WW], in0=EA[:, :WW],
                                        in1=band[:, :WW], op=AT.mult)
                rsum = gp.tile([128, 1], F32, name="rsum", tag="rsum", bufs=1)
                nc.vector.tensor_reduce(out=rsum[:], in_=EW[:, :WW],
                                        axis=AX.X, op=AT.add)
                rinv = gp.tile([128, 1], F32, name="rinv", tag="rinv", bufs=1)
                nc.vector.reciprocal(rinv[:], rsum[:])
                wg = gp.tile([128, K_WIN], BF, name="wg", tag="wg", bufs=2)
                nc.vector.tensor_scalar(out=wg[:, :WW], in0=EW[:, :WW],
                                        scalar1=rinv[:, 0:1], scalar2=None,
                                        op0=AT.mult)

                # PE-transpose wg → wgT [token, span]
                wgT = gp.tile([128, K_WIN], BF, name="wgT", tag="wgT", bufs=2)
                for kk in range(KCg):
                    trp = ps.tile([128, 128], BF, name="tr", tag="tr", bufs=2)
                    nc.tensor.transpose(
                        trp[:], wg[:, kk * 128 : (kk + 1) * 128], ident_t[:])
                    nc.scalar.copy(wgT[:, kk * 128 : (kk + 1) * 128], trp[:])

                # flipped gather: h1[span, hid] += OH.T @ Pwin, N=512
                hp = ps.tile([128, HID], F32, name="hp", tag="hp", bufs=2)
                steps = []
                for kk in range(KCg):
                    bs = slice((boff[g] + kk) * 128, (boff[g] + kk + 1) * 128)
                    steps.append((ohs_t[:, bs], pw[0][kk]))
                    steps.append((ohe_t[:, bs], pw[1][kk]))
                steps.append((ohl_t[:, g * 128 : (g + 1) * 128], WB_t))
                for kk in range(KCg):
                    steps.append((wgT[:, kk * 128 : (kk + 1) * 128], pw[2][kk]))
                for h0 in (0, 512):
                    for i, (lhsT, rhs) in enumerate(steps):
                        nc.tensor.matmul(hp[:, h0 : h0 + 512], lhsT,
                                         rhs[:, h0 : h0 + 512],
                                         start=(i == 0),
                                         stop=(i == len(steps) - 1))

                if pend is not None:
                    emit_relu_transpose(pend)
                if g % 4 == 0 and g > 0:
                    emit_l2_l3(h1b_prev, (g // 4 - 1) * 512)
                if g == NGROUPS - 2:
                    # first half of the final block (groups 12-13) early, so
                    # only a half-width L2/L3 remains after the last gather
                    emit_l2_l3(h1b_cur, (NGROUPS // 4 - 1) * 512, 0, 256)
                h1 = gp.tile([128, HID], BF, name="h1", tag="h1", bufs=2)
                pend = (hp, h1, gcol, h1b_cur)

            emit_relu_transpose(pend)
            emit_l2_l3(h1b_cur, (NGROUPS // 4 - 1) * 512, 256, 256)

    if SPLIT_WAITS:
        _split_waits(nc)
    return nc


def _split_waits(nc, max_waits=1):
    """This walrus build rejects instructions carrying >max_waits sem waits
    ("Too many sync wait commands"). Hoist excess waits onto same-engine
    NoOps placed immediately before the instruction — identical semantics
    (engine queues are in-order)."""
    ctr = [0]
    for f in nc.m.functions:
        for blk in f.blocks:
            out = []
            for ins in blk.instructions:
                si = getattr(ins, "sync_info", None)
                if si is not None and si.on_wait and len(si.on_wait) > max_waits:
                    waits = list(si.on_wait)
                    for w in waits[:-max_waits]:
                        ctr[0] += 1
                        nop = mybir.InstNoOp(
                            name=f"I-wsplit-{ctr[0]}", ins=[], outs=[],
                            sync_info=mybir.SyncInfo(on_wait=[w], on_update=[]),
                        )
                        nop.engine = ins.engine
                        out.append(nop)
                    ins.sync_info = mybir.SyncInfo(
                        on_wait=waits[-max_waits:],
                        on_update=list(si.on_update or []),
                    )
                out.append(ins)
            blk.instructions[:] = out
    return ctr[0]


_CACHE = {}
LAST_EXEC_NS = None
TRACE = False


def _install_ntff_shim():
    try:
        import antenv.axon_hooks  # noqa: F401
        return
    except ImportError:
        pass
    try:
        from trn_agent_boot.trn_boot import _ntff_profile_via_ctypes
        hook = _ntff_profile_via_ctypes("/opt/axon/libaxon_pjrt.so")
    except Exception:
        hook = None
    m1 = types.ModuleType("antenv")
    m2 = types.ModuleType("antenv.axon_hooks")
    m2.get_axon_ntff_profile_hook = lambda: hook
    m2.set_axon_ntff_profile_hook = lambda h: None
    m1.axon_hooks = m2
    sys.modules.setdefault("antenv", m1)
    sys.modules["antenv.axon_hooks"] = m2


def _prepare(inputs):
    inp = {k: np.asarray(v) for k, v in inputs.items()}
    ss = inp["span_starts"].astype(np.int64)
    sl = inp["span_lengths"].astype(np.int64)
    plan = _plan(ss, sl)
    T_cap, K_WIN, bases = plan["T_cap"], plan["K_WIN"], plan["bases"]
    KC = K_WIN // 128
    b3val = float(np.asarray(inp["score_b3"]).reshape(-1)[0])

    kcs = plan["kcs"]
    key = (T_cap, K_WIN, tuple(bases), tuple(kcs), b3val)
    if key not in _CACHE:
        _CACHE[key] = _build(T_cap, K_WIN, bases, kcs, b3val)
    nc = _CACHE[key]

    def bfc(x):
        return np.ascontiguousarray(np.asarray(x, dtype=np.float32)).astype(bf16)

    sw1 = inp["score_w1"].astype(np.float32)

    def f8pack(w):
        # [1024,1024] -> [128, ktile, m] fp8, pre-scaled by WSCALE
        a = (np.asarray(w, np.float32) * WSCALE).reshape(8, 128, HID)
        return np.ascontiguousarray(
            a.transpose(1, 0, 2).reshape(128, 8 * HID)).astype(f8e4)

    shared = {
        "aw1f": f8pack(inp["attn_w1"]),
        "aw2f": f8pack(inp["attn_w2"]),
        "aw3m": bfc(inp["attn_w3"].reshape(8, 128).T),
        "ab1m": np.ascontiguousarray(
            inp["attn_b1"].astype(np.float32).reshape(8, 128).T),
        "ab2m": np.ascontiguousarray(
            inp["attn_b2"].astype(np.float32).reshape(8, 128).T),
        "w1a": bfc(sw1[0:1024]),
        "w1b": bfc(sw1[1024:2048]),
        "w1c": bfc(sw1[2048:3072]),
        "w2": bfc(inp["score_w2"]),
        "b2m": np.ascontiguousarray(
            inp["score_b2"].astype(np.float32).reshape(8, 128).T),
        "w3m": bfc(inp["score_w3"].reshape(8, 128).T),
        "w1d": bfc(sw1[3072:3092]),
        "wtT": bfc(inp["width_table"].T),
        "b1r": bfc(inp["score_b1"].reshape(1, HID)),
        "iotaW": np.arange(K_WIN, dtype=np.float32).reshape(1, -1),
        "ident": np.eye(128, dtype=np.float32).astype(bf16),
    }

    states = inp["states"].astype(np.float32)
    embeds = inp["embeds"].astype(np.float32)
    in_maps = []
    for c in range(N_CORES):
        cb = int(plan["core_base"][c])
        stl = np.zeros((T_cap, D), np.float32)
        eml = np.zeros((T_cap, D), np.float32)
        hi = min(T, cb + T_cap)
        stl[: hi - cb] = states[cb:hi]
        eml[: hi - cb] = embeds[cb:hi]
        m = dict(shared)
        m["statesT"] = np.ascontiguousarray(stl.T).astype(bf16)
        m["statesTf"] = np.ascontiguousarray(stl.T).astype(f8e4)
        m["embedsT"] = np.ascontiguousarray(eml.T).astype(bf16)
        d = plan["d"][c].astype(np.float32)
        dl = plan["dl"][c].astype(np.float32)
        ln = plan["ln"][c].astype(np.int64)
        m["dmat"] = np.ascontiguousarray(d.reshape(G, 128).T)
        m["demat"] = np.ascontiguousarray(dl.reshape(G, 128).T)
        # host-built one-hot gather matrices, [token, span] layout
        NBLK = sum(kcs)
        boff = np.cumsum([0] + kcs)
        di = plan["d"][c].astype(np.int64).reshape(G, 128)
        dei = plan["dl"][c].astype(np.int64).reshape(G, 128)
        ohs = np.zeros((128, NBLK * 128), np.float32)
        ohe = np.zeros((128, NBLK * 128), np.float32)
        cols = np.arange(128)
        for g in range(G):
            for kk in range(kcs[g]):
                c0 = (boff[g] + kk) * 128
                for arr, idx in ((ohs, di[g]), (ohe, dei[g])):
                    r = idx - kk * 128
                    sel = (r >= 0) & (r < 128)
                    arr[r[sel], c0 + cols[sel]] = 1.0
        m["ohs"] = ohs.astype(bf16)
        m["ohe"] = ohe.astype(bf16)
        ohl = np.zeros((128, C), np.float32)
        ohl[ln, np.arange(C)] = 1.0
        m["ohl"] = ohl.astype(bf16)
        in_maps.append(m)

    return nc, in_maps, plan


def kernel(**inputs):
    global LAST_EXEC_NS
    from concourse.bass_utils import run_bass_kernel_spmd

    nc, in_maps, plan = _prepare(inputs)
    _install_ntff_shim()
    res = run_bass_kernel_spmd(nc, in_maps, list(range(N_CORES)), trace=TRACE)
    LAST_EXEC_NS = res.exec_time_ns

    out = np.empty(NSPAN, np.float32)
    for c in range(N_CORES):
        out[plan["order"][c * C : (c + 1) * C]] = np.asarray(
            res.results[c]["scores"]).reshape(-1)
    return out.reshape(NSPAN, 1)



# revision 31
# speedup vs baseline: 1.0720x; 1.0267x over previous
"""Trainium2 Bass kernel for nn_MentionScore.

Strategy: sort spans by start, shard 2048 consecutive sorted spans per core.
Each core only touches a ~1.1k-token window of states/embeds (host passes the
window pre-transposed, bf16). The ragged gather/softmax/weighted-sum becomes
dense matmuls against one-hot / banded matrices built on-device with
iota-compare tensor ops. Layer-1 of the span MLP is algebraically folded:
  h1 = relu(OH_s.T@P1 + OH_e.T@P2 + Wg.T@P3 + onehot(len).T@WB)
with P1=states@W1a, P2=states@W1b, P3=embeds@W1c precomputed per token and
WB = width_table@W1d + b1.
"""

import sys
import types

import numpy as np
import ml_dtypes

import concourse.bass as bass
import concourse.mybir as mybir
from concourse.ap import AP
from concourse.tile import TileContext
from concourse.vector_clock import ScopedClock

BF = mybir.dt.bfloat16
F32 = mybir.dt.float32
F8 = mybir.dt.float8e4
PM = mybir.MatmulPerfMode
AT = mybir.AluOpType
AF = mybir.ActivationFunctionType
AX = mybir.AxisListType
bf16 = ml_dtypes.bfloat16
f8e4 = ml_dtypes.float8_e4m3
WSCALE = 16.0

N_CORES = 8
T, NSPAN, D, HID, LMAX, WD = 8192, 16384, 1024, 1024, 10, 20
C = NSPAN // N_CORES          # spans per core
G = C // 128                  # 128-span groups per core


class PatchedTileContext(TileContext):
    """Workaround: walrus rejects the tail Drain when it carries >1 sem wait
    ("Too many sync wait commands"). Put each wait on its own NoOp instead."""

    def _drain_and_barrier(self, tick_clock, wait_clock):
        nc = self.nc
        drain_inst = nc.sync.drain()
        wait_clock.add_sem_waits(
            drain_inst.ins, ScopedClock({None: tick_clock.global_clock})
        )
        si = drain_inst.ins.sync_info
        if si is not None and si.on_wait is not None and len(si.on_wait) > 1:
            waits = list(si.on_wait)
            drain_inst.ins.sync_info = mybir.SyncInfo(
                on_wait=[waits[0]], on_update=list(si.on_update or [])
            )
            for w in waits[1:]:
                nop = nc.sync.nop()
                nop.ins.sync_info = mybir.SyncInfo(on_wait=[w], on_update=[])

        nc.all_engine_barrier()
        assert self.sems is not None
        popped = nc._tile_sem_poison_stack.pop()
        assert popped is self._sem_poison
        nc.clear_and_free_semaphores(list(self.sems.allocated().values()))
        nc.all_engine_barrier()


def _ceil128(x):
    return int(-(-int(x) // 128) * 128)


def _plan(span_starts, span_lengths):
    """Host-side sharding plan. Returns per-core data + static layout consts."""
    order = np.argsort(span_starts, kind="stable").astype(np.int64)
    ss = span_starts[order].reshape(N_CORES, C).astype(np.int64)
    sl = span_lengths[order].reshape(N_CORES, C).astype(np.int64)
    core_base = ss[:, 0].copy()
    sloc = ss - core_base[:, None]
    eloc = sloc + sl

    T_cap = _ceil128(int(eloc.max()) + 1)
    # unaligned, shared-across-cores group window bases + per-group k-tiles
    mn = sloc[:, ::128].min(axis=0)                       # [G]
    mx = eloc.reshape(N_CORES, G, 128).max(axis=2).max(axis=0)  # [G]
    need = mx - mn + 1
    kcs = np.maximum((need + 127) // 128, 1)
    T_pad = T_cap + 128
    bases = mn.copy()
    for _ in range(3):
        bases = np.minimum(mn, T_pad - kcs * 128)
        bad = (mx - bases + 1) > kcs * 128
        if not bad.any():
            break
        kcs[bad] += 1
    K_WIN = int(kcs.max()) * 128
    d = sloc - np.repeat(bases, 128)[None, :]
    assert d.min() >= 0 and ((d + sl).reshape(N_CORES, G, 128).max(axis=2)
                             <= kcs[None, :] * 128 - 1).all(), "window overflow"

    return {
        "order": order,
        "core_base": core_base,
        "sloc": sloc,
        "d": d.astype(np.float64),
        "dl": (d + sl).astype(np.float64),
        "ln": sl.astype(np.float64),
        "T_cap": T_cap,
        "K_WIN": int(K_WIN),
        "bases": [int(b) for b in bases],
        "kcs": [int(k) for k in kcs],
    }


NGROUPS = G
SPLIT_WAITS = True


def _build(T_cap, K_WIN, bases, kcs, b3val):
    """Build the single SPMD Bass program (static; shared by all 8 cores)."""
    TC = T_cap // 128
    KC = K_WIN // 128
    T_pad = T_cap + 128
    nc = bass.Bass()

    def par(name, shape, dt):
        return nc.declare_dram_parameter(name, list(shape), dt, isOutput=False)

    NBLK = sum(kcs)
    boff = [0]
    for k in kcs:
        boff.append(boff[-1] + k)

    statesT_p = par("statesT", [D, T_cap], BF)
    embedsT_p = par("embedsT", [D, T_cap], BF)
    dmat_p = par("dmat", [128, G], F32)
    demat_p = par("demat", [128, G], F32)
    ohs_p = par("ohs", [128, NBLK * 128], BF)
    ohe_p = par("ohe", [128, NBLK * 128], BF)
    statesTf_p = par("statesTf", [D, T_cap], F8)
    aw1f_p = par("aw1f", [128, 8 * HID], F8)
    aw2f_p = par("aw2f", [128, 8 * HID], F8)
    aw3_p = par("aw3m", [128, 8], BF)
    ab1_p = par("ab1m", [128, 8], F32)
    ab2_p = par("ab2m", [128, 8], F32)
    w1a_p = par("w1a", [D, HID], BF)
    w1b_p = par("w1b", [D, HID], BF)
    w1c_p = par("w1c", [D, HID], BF)
    w1d_p = par("w1d", [WD, HID], BF)
    wtT_p = par("wtT", [WD, LMAX], BF)
    b1r_p = par("b1r", [1, HID], BF)
    ohl_p = par("ohl", [128, C], BF)
    w2_p = par("w2", [HID, HID], BF)
    b2_p = par("b2m", [128, 8], F32)
    w3_p = par("w3m", [128, 8], BF)
    iotaW_p = par("iotaW", [1, K_WIN], F32)
    ident_p = par("ident", [128, 128], BF)
    scores_p = nc.declare_dram_parameter("scores", [1, C], F32, isOutput=True)

    with PatchedTileContext(nc) as tc:
        with (
            tc.tile_pool(name="pp", bufs=1) as pp,
            tc.tile_pool(name="wst", bufs=2) as wst,
            tc.tile_pool(name="gp", bufs=2) as gp,
            tc.tile_pool(name="ps", bufs=2, space="PSUM") as ps,
            tc.tile_pool(name="dp", bufs=1, space="DRAM") as dp,
        ):
            dma = nc.sync.dma_start
            nblocks = [(n0, min(512, T_cap - n0))
                       for n0 in range(0, T_cap, 512)]

            def load_sTw8(n0, nw):
                t = wst.tile([128, 8, 512], F8, name="sTw8", tag="sTw8",
                             bufs=2)
                for k in range(8):
                    dma(out=t[:, k : k + 1, :nw],
                        in_=statesTf_p[k * 128 : (k + 1) * 128, n0 : n0 + nw])
                return t

            def load_tok(param, pfx, n0, nw, bufs):
                tiles = []
                for k in range(8):
                    t = wst.tile([128, 512], BF, name=f"{pfx}{k}",
                                 tag=f"{pfx}{k}", bufs=bufs)
                    dma(out=t[:, :nw],
                        in_=param[k * 128 : (k + 1) * 128, n0 : n0 + nw])
                    tiles.append(t)
                return tiles

            # critical path first: fp8 attn weights + block-0 tokens
            aw1f_t = pp.tile([128, 8, HID], F8, name="aw1f", tag="aw1f")
            dma(out=aw1f_t[:], in_=aw1f_p[:])
            sTw8_0 = load_sTw8(*nblocks[0])

            # ---------- constants / scalars ----------
            iotaW_t = pp.tile([128, K_WIN], F32, name="iotaW", tag="iotaW")
            dma(out=iotaW_t[:], in_=iotaW_p[:].partition_broadcast(128))
            ident_t = pp.tile([128, 128], BF, name="ident", tag="ident")
            dma(out=ident_t[:], in_=ident_p[:])
            ones16_t = pp.tile([1, 16], BF, name="ones16", tag="ones16")
            nc.vector.memset(ones16_t[:], 1.0)
            dmat_t = pp.tile([128, G], F32, name="dmat", tag="dmat")
            dma(out=dmat_t[:], in_=dmat_p[:])
            demat_t = pp.tile([128, G], F32, name="demat", tag="demat")
            dma(out=demat_t[:], in_=demat_p[:])
            ab1_t = pp.tile([128, 8], F32, name="ab1", tag="ab1")
            dma(out=ab1_t[:], in_=ab1_p[:])
            ab2_t = pp.tile([128, 8], F32, name="ab2", tag="ab2")
            dma(out=ab2_t[:], in_=ab2_p[:])
            b2_t = pp.tile([128, 8], F32, name="b2", tag="b2")
            dma(out=b2_t[:], in_=b2_p[:])
            aw3_t = pp.tile([128, 8], BF, name="aw3", tag="aw3")
            dma(out=aw3_t[:], in_=aw3_p[:])
            w3_t = pp.tile([128, 8], BF, name="w3", tag="w3")
            dma(out=w3_t[:], in_=w3_p[:])
            b1r_t = pp.tile([1, HID], BF, name="b1r", tag="b1r")
            dma(out=b1r_t[:], in_=b1r_p[:])
            w1d_t = pp.tile([WD, HID], BF, name="w1d", tag="w1d")
            dma(out=w1d_t[:], in_=w1d_p[:])
            wtT_t = pp.tile([WD, 16], BF, name="wtT", tag="wtT")
            nc.vector.memset(wtT_t[:], 0.0)
            dma(out=wtT_t[:, :LMAX], in_=wtT_p[:])

            attns_dram = dp.tile(
                [T_pad + 16], BF, name="attns_dram", tag="attns_dram")

            if NGROUPS < G:  # debug builds: ensure output is written
                zsc = pp.tile([1, C], F32, name="zsc", tag="zsc")
                nc.vector.memset(zsc[:], 0.0)
                dma(out=scores_p[:], in_=zsc[:])

            # ---------- weight slots ----------
            def wload(param, tag_prefix):
                tiles = []
                for k in range(8):
                    t = pp.tile([128, HID], BF, name=f"{tag_prefix}{k}",
                                tag=f"{tag_prefix}{k}")
                    dma(out=t[:], in_=param[k * 128 : (k + 1) * 128, :])
                    tiles.append(t)
                return tiles

            aw2f_t = pp.tile([128, 8, HID], F8, name="aw2f", tag="aw2f")
            dma(out=aw2f_t[:], in_=aw2f_p[:])
            w1a_t = wload(w1a_p, "wWA")
            sTw_0 = load_tok(statesT_p, "sTw", nblocks[0][0], nblocks[0][1], 2)
            w1b_t = wload(w1b_p, "wWB")
            w1c_t = wload(w1c_p, "wWC")
            eTw_0 = load_tok(embedsT_p, "eTw", nblocks[0][0], nblocks[0][1], 2)

            # ---------- P targets in DRAM, split at row PSPLIT so early
            # group windows stop depending on the tail of the projection ----
            PSPLIT = 640
            PdA = [dp.tile([PSPLIT, HID], BF, name=f"P{i}a", tag=f"P{i}a")
                   for i in range(3)]
            PdB = [dp.tile([T_pad - PSPLIT, HID], BF, name=f"P{i}b",
                           tag=f"P{i}b") for i in range(3)]
            zrow = pp.tile([128, HID], BF, name="zrow", tag="zrow")
            nc.vector.memset(zrow[:], 0.0)
            for pd in PdB:
                dma(out=pd[T_cap - PSPLIT :, :], in_=zrow[:])

            def p_write(pi, r0, h0, src_ap):
                # r0 is 128-aligned so a chunk never straddles PSPLIT
                if r0 < PSPLIT:
                    dma(out=PdA[pi][r0 : r0 + 128, h0 : h0 + 512], in_=src_ap)
                else:
                    dma(out=PdB[pi][r0 - PSPLIT : r0 - PSPLIT + 128,
                                    h0 : h0 + 512], in_=src_ap)

            def p_read(pi, r0, dst):
                # window rows [r0, r0+128) may straddle PSPLIT
                if r0 + 128 <= PSPLIT:
                    dma(out=dst[:], in_=PdA[pi][r0 : r0 + 128, :])
                elif r0 >= PSPLIT:
                    dma(out=dst[:], in_=PdB[pi][r0 - PSPLIT : r0 - PSPLIT + 128, :])
                else:
                    rr = PSPLIT - r0
                    dma(out=dst[:rr, :], in_=PdA[pi][r0:PSPLIT, :])
                    dma(out=dst[rr:, :], in_=PdB[pi][: 128 - rr, :])

            # ---------- blocked token pipeline: attn MLP + P projections --
            attns_t = pp.tile([1, T_cap], BF, name="attns", tag="attns")

            def emit_P(sTw, eTw, n0, nw):
                for j in range(nw // 128):
                    js = slice(j * 128, (j + 1) * 128)
                    for pi, (wt_, srcs) in enumerate(
                            ((w1a_t, sTw), (w1b_t, sTw), (w1c_t, eTw))):
                        for h0 in (0, 512):
                            pt = ps.tile([128, 512], F32, name="big",
                                         tag="big", bufs=2)
                            for k in range(8):
                                nc.tensor.matmul(
                                    pt[:], srcs[k][:, js],
                                    wt_[k][:, h0 : h0 + 512],
                                    start=(k == 0), stop=(k == 7))
                            stg = wst.tile([128, 512], BF, name=f"pstg{pi}",
                                           tag=f"pstg{pi}", bufs=2)
                            nc.scalar.copy(stg[:], pt[:])
                            p_write(pi, n0 + j * 128, h0, stg[:])

            pend_P = None
            for bi, (n0, nw) in enumerate(nblocks):
                sTw8 = sTw8_0 if bi == 0 else load_sTw8(n0, nw)
                sTw = sTw_0 if bi == 0 else load_tok(statesT_p, "sTw", n0, nw, 2)
                eTw = eTw_0 if bi == 0 else load_tok(embedsT_p, "eTw", n0, nw, 2)
                h1a8 = wst.tile([128, 8, 512], F8, name="h1a8", tag="h1a8",
                                bufs=1)
                h2a = [wst.tile([128, 512], BF, name=f"h2a{h}", tag=f"h2a{h}", bufs=1)
                       for h in range(8)]
                for hc in range(8):
                    pt = ps.tile([128, 512], F32, name="big", tag="big", bufs=2)
                    for p in range(4):
                        nc.tensor.matmul(
                            pt[:, :nw],
                            aw1f_t[:, 2 * p : 2 * p + 2,
                                   hc * 128 : (hc + 1) * 128],
                            sTw8[:, 2 * p : 2 * p + 2, :nw],
                            start=(p == 0), stop=(p == 3),
                            perf_mode=PM.DoubleRow)
                    nc.scalar.activation(
                        h1a8[:, hc : hc + 1, :nw], pt[:, :nw], AF.Relu,
                        bias=ab1_t[:, hc : hc + 1], scale=1.0 / WSCALE)
                for hc in range(8):
                    pt = ps.tile([128, 512], F32, name="big", tag="big", bufs=2)
                    for p in range(4):
                        nc.tensor.matmul(
                            pt[:, :nw],
                            aw2f_t[:, 2 * p : 2 * p + 2,
                                   hc * 128 : (hc + 1) * 128],
                            h1a8[:, 2 * p : 2 * p + 2, :nw],
                            start=(p == 0), stop=(p == 3),
                            perf_mode=PM.DoubleRow)
                    nc.scalar.activation(
                        h2a[hc][:, :nw], pt[:, :nw], AF.Relu,
                        bias=ab2_t[:, hc : hc + 1], scale=1.0 / WSCALE)
                pt = ps.tile([1, 512], F32, name="big", tag="big", bufs=2)
                for k in range(8):
                    nc.tensor.matmul(
                        pt[:, :nw], aw3_t[:, k : k + 1], h2a[k][:, :nw],
                        start=(k == 0), stop=(k == 7))
                nc.scalar.activation(attns_t[:, n0 : n0 + nw], pt[:, :nw],
                                     AF.Copy)
                dma(out=attns_dram[n0 : n0 + nw], in_=attns_t[0:1, n0 : n0 + nw])
                emit_P(sTw, eTw, n0, nw)

            zpad_t = pp.tile([1, 144], BF, name="zpad", tag="zpad")
            nc.vector.memset(zpad_t[:], 0.0)
            dma(out=attns_dram[T_cap:], in_=zpad_t[0:1, :])

            # ---- WB = width_table @ W1d + b1 → [128, HID] bf16 (zero-pad)
            WB_t = pp.tile([128, HID], BF, name="WB", tag="WB")
            nc.vector.memset(WB_t[:], 0.0)
            for n0 in range(0, HID, 512):
                pt = ps.tile([16, 512], F32, name="big", tag="big", bufs=2)
                nc.tensor.matmul(pt[:], wtT_t[:], w1d_t[:, n0 : n0 + 512],
                                 start=True, stop=False)
                nc.tensor.matmul(pt[:], ones16_t[:], b1r_t[:, n0 : n0 + 512],
                                 start=False, stop=True)
                nc.vector.tensor_copy(out=WB_t[:16, n0 : n0 + 512], in_=pt[:])

            # ---------- span groups ----------
            w2_t = wload(w2_p, "wA")     # reuse again for L2
            # one-hot gather matrices, precomputed on host ([token, span])
            ohs_t = pp.tile([128, NBLK * 128], BF, name="ohs", tag="ohs")
            dma(out=ohs_t[:], in_=ohs_p[:])
            ohe_t = pp.tile([128, NBLK * 128], BF, name="ohe", tag="ohe")
            dma(out=ohe_t[:], in_=ohe_p[:])
            ohl_t = pp.tile([128, C], BF, name="ohl", tag="ohl")
            dma(out=ohl_t[:], in_=ohl_p[:])

            # Software pipeline: iteration g emits softmax+gather for group g
            # and the (relu, h1-transpose) for group g-1; the L2/L3 block for
            # groups 4b..4b+3 is emitted at the start of iteration 4b+4.
            h1b_cur = h1b_prev = None
            pend = None          # (hp, h1, gcol, h1b) awaiting relu+transpose

            def emit_relu_transpose(pend):
                hp, h1, gcol, h1b = pend
                for h0 in (0, 512):
                    nc.scalar.activation(h1[:, h0 : h0 + 512],
                                         hp[:, h0 : h0 + 512], AF.Relu)
                for hc in range(8):
                    trp = ps.tile([128, 128], BF, name="tr", tag="tr", bufs=2)
                    nc.tensor.transpose(
                        trp[:], h1[:, hc * 128 : (hc + 1) * 128], ident_t[:])
                    nc.vector.tensor_copy(
                        out=h1b[hc][:, gcol : gcol + 128], in_=trp[:])

            def emit_l2_l3(h1b, b0, c0=0, cw=512):
                cs = slice(c0, c0 + cw)
                h2b = [gp.tile([128, 512], BF, name=f"h2b{k}", tag=f"h2b{k}",
                               bufs=1) for k in range(8)]
                for h2c in range(8):
                    pt = ps.tile([128, 512], F32, name="big", tag="big", bufs=2)
                    for k in range(8):
                        nc.tensor.matmul(
                            pt[:, :cw], w2_t[k][:, h2c * 128 : (h2c + 1) * 128],
                            h1b[k][:, cs], start=(k == 0), stop=(k == 7))
                    nc.scalar.activation(h2b[h2c][:, :cw], pt[:, :cw], AF.Relu,
                                         bias=b2_t[:, h2c : h2c + 1])
                pt = ps.tile([1, 512], F32, name="big", tag="big", bufs=2)
                for k in range(8):
                    nc.tensor.matmul(pt[:, :cw], w3_t[:, k : k + 1],
                                     h2b[k][:, :cw],
                                     start=(k == 0), stop=(k == 7))
                ob = gp.tile([1, 512], F32, name="ob", tag="ob")
                nc.scalar.activation(ob[:, :cw], pt[:, :cw], AF.Copy,
                                     bias=float(b3val))
                dma(out=scores_p[:, b0 + c0 : b0 + c0 + cw], in_=ob[:, :cw])

            for g in range(NGROUPS):
                KCg = kcs[g]
                WW = KCg * 128
                if g % 4 == 0:
                    h1b_prev = h1b_cur
                    h1b_cur = [gp.tile([128, 512], BF, name=f"h1b{k}",
                                       tag=f"h1b{k}", bufs=2) for k in range(8)]
                gcol = (g % 4) * 128

                # P windows from DRAM; reuse the idle w1a/b/c weight slots as
                # a manual double buffer (alternating on g%2).
                pw = []
                for pi, pfx in enumerate(("wWA", "wWB", "wWC")):
                    tiles = []
                    for kk in range(KCg):
                        pt_ = pp.tile([128, HID], BF,
                                      name=f"{pfx}{kk * 4 + g % 4}",
                                      tag=f"{pfx}{kk * 4 + g % 4}")
                        p_read(pi, bases[g] + kk * 128, pt_)
                        tiles.append(pt_)
                    pw.append(tiles)

                # attns window, broadcast to all partitions: A[n, t]
                A_t = gp.tile([128, K_WIN], BF, name="A", tag="A", bufs=2)
                dma(out=A_t[:, :WW],
                    in_=attns_dram[bases[g] : bases[g] + WW]
                    .partition_broadcast(128))
                # exp early on scalar queue (before prev group's relu)
                EA = gp.tile([128, K_WIN], F32, name="EA", tag="EA", bufs=2)
                nc.scalar.activation(EA[:, :WW], A_t[:, :WW], AF.Exp)

                # band mask d[n] <= t <= d[n]+len[n], softmax over the band
                m1 = gp.tile([128, K_WIN], F32, name="m1", tag="m1", bufs=1)
                nc.vector.tensor_scalar(out=m1[:, :WW], in0=iotaW_t[:, :WW],
                                        scalar1=dmat_t[:, g : g + 1],
                                        scalar2=None, op0=AT.is_ge)
                m2 = gp.tile([128, K_WIN], F32, name="m2", tag="m2", bufs=1)
                nc.vector.tensor_scalar(out=m2[:, :WW], in0=iotaW_t[:, :WW],
                                        scalar1=demat_t[:, g : g + 1],
                                        scalar2=None, op0=AT.is_le)
                band = gp.tile([128, K_WIN], F32, name="band", tag="band", bufs=1)
                nc.vector.tensor_tensor(out=band[:, :WW], in0=m1[:, :WW],
                                        in1=m2[:, :WW], op=AT.mult)
                EW = gp.tile([128, K_WIN], F32, name="EW", tag="EW", bufs=1)
                nc.vector.tensor_tensor(out=EW[:, :WW], in0=EA[:, :WW],
                                        in1=band[:, :WW], op=AT.mult)
                rsum = gp.tile([128, 1], F32, name="rsum", tag="rsum", bufs=1)
                nc.vector.tensor_reduce(out=rsum[:], in_=EW[:, :WW],
                                        axis=AX.X, op=AT.add)
                rinv = gp.tile([128, 1], F32, name="rinv", tag="rinv", bufs=1)
                nc.vector.reciprocal(rinv[:], rsum[:])
                wg = gp.tile([128, K_WIN], BF, name="wg", tag="wg", bufs=2)
                nc.vector.tensor_scalar(out=wg[:, :WW], in0=EW[:, :WW],
                                        scalar1=rinv[:, 0:1], scalar2=None,
                                        op0=AT.mult)

                # PE-transpose wg → wgT [token, span]
                wgT = gp.tile([128, K_WIN], BF, name="wgT", tag="wgT", bufs=2)
                for kk in range(KCg):
                    trp = ps.tile([128, 128], BF, name="tr", tag="tr", bufs=2)
                    nc.tensor.transpose(
                        trp[:], wg[:, kk * 128 : (kk + 1) * 128], ident_t[:])
                    nc.scalar.copy(wgT[:, kk * 128 : (kk + 1) * 128], trp[:])

                # flipped gather: h1[span, hid] += OH.T @ Pwin, N=512
                hp = ps.tile([128, HID], F32, name="hp", tag="hp", bufs=2)
                steps = []
                for kk in range(KCg):
                    bs = slice((boff[g] + kk) * 128, (boff[g] + kk + 1) * 128)
                    steps.append((ohs_t[:, bs], pw[0][kk]))
                    steps.append((ohe_t[:, bs], pw[1][kk]))
                steps.append((ohl_t[:, g * 128 : (g + 1) * 128], WB_t))
                for kk in range(KCg):
                    steps.append((wgT[:, kk * 128 : (kk + 1) * 128], pw[2][kk]))
                for h0 in (0, 512):
                    for i, (lhsT, rhs) in enumerate(steps):
                        nc.tensor.matmul(hp[:, h0 : h0 + 512], lhsT,
                                         rhs[:, h0 : h0 + 512],
                                         start=(i == 0),
                                         stop=(i == len(steps) - 1))

                if pend is not None:
                    emit_relu_transpose(pend)
                if g % 4 == 0 and g > 0:
                    emit_l2_l3(h1b_prev, (g // 4 - 1) * 512)
                if g == NGROUPS - 2:
                    # first half of the final block (groups 12-13) early, so
                    # only a half-width L2/L3 remains after the last gather
                    emit_l2_l3(h1b_cur, (NGROUPS // 4 - 1) * 512, 0, 256)
                h1 = gp.tile([128, HID], BF, name="h1", tag="h1", bufs=2)
                pend = (hp, h1, gcol, h1b_cur)

            emit_relu_transpose(pend)
            emit_l2_l3(h1b_cur, (NGROUPS // 4 - 1) * 512, 256, 256)

    if SPLIT_WAITS:
        _split_waits(nc)
    return nc


def _split_waits(nc, max_waits=1):
    """This walrus build rejects instructions carrying >max_waits sem waits
    ("Too many sync wait commands"). Hoist excess waits onto same-engine
    NoOps placed immediately before the instruction — identical semantics
    (engine queues are in-order)."""
    ctr = [0]
    for f in nc.m.functions:
        for blk in f.blocks:
            out = []
            for ins in blk.instructions:
                si = getattr(ins, "sync_info", None)
                if si is not None and si.on_wait and len(si.on_wait) > max_waits:
                    waits = list(si.on_wait)
                    for w in waits[:-max_waits]:
                        ctr[0] += 1
                        nop = mybir.InstNoOp(
                            name=f"I-wsplit-{ctr[0]}", ins=[], outs=[],
                            sync_info=mybir.SyncInfo(on_wait=[w], on_update=[]),
                        )
                        nop.engine = ins.engine
                        out.append(nop)
                    ins.sync_info = mybir.SyncInfo(
                        on_wait=waits[-max_waits:],
                        on_update=list(si.on_update or []),
                    )
                out.append(ins)
            blk.instructions[:] = out
    return ctr[0]


_CACHE = {}
LAST_EXEC_NS = None
TRACE = False


def _install_ntff_shim():
    try:
        import antenv.axon_hooks  # noqa: F401
        return
    except ImportError:
        pass
    try:
        from trn_agent_boot.trn_boot import _ntff_profile_via_ctypes
        hook = _ntff_profile_via_ctypes("/opt/axon/libaxon_pjrt.so")
    except Exception:
        hook = None
    m1 = types.ModuleType("antenv")
    m2 = types.ModuleType("antenv.axon_hooks")
    m2.get_axon_ntff_profile_hook = lambda: hook
    m2.set_axon_ntff_profile_hook = lambda h: None
    m1.axon_hooks = m2
    sys.modules.setdefault("antenv", m1)
    sys.modules["antenv.axon_hooks"] = m2


def _prepare(inputs):
    inp = {k: np.asarray(v) for k, v in inputs.items()}
    ss = inp["span_starts"].astype(np.int64)
    sl = inp["span_lengths"].astype(np.int64)
    plan = _plan(ss, sl)
    T_cap, K_WIN, bases = plan["T_cap"], plan["K_WIN"], plan["bases"]
    KC = K_WIN // 128
    b3val = float(np.asarray(inp["score_b3"]).reshape(-1)[0])

    kcs = plan["kcs"]
    key = (T_cap, K_WIN, tuple(bases), tuple(kcs), b3val)
    if key not in _CACHE:
        _CACHE[key] = _build(T_cap, K_WIN, bases, kcs, b3val)
    nc = _CACHE[key]

    def bfc(x):
        return np.ascontiguousarray(np.asarray(x, dtype=np.float32)).astype(bf16)

    sw1 = inp["score_w1"].astype(np.float32)

    def f8pack(w):
        # [1024,1024] -> [128, ktile, m] fp8, pre-scaled by WSCALE
        a = (np.asarray(w, np.float32) * WSCALE).reshape(8, 128, HID)
        return np.ascontiguousarray(
            a.transpose(1, 0, 2).reshape(128, 8 * HID)).astype(f8e4)

    shared = {
        "aw1f": f8pack(inp["attn_w1"]),
        "aw2f": f8pack(inp["attn_w2"]),
        "aw3m": bfc(inp["attn_w3"].reshape(8, 128).T),
        "ab1m": np.ascontiguousarray(
            inp["attn_b1"].astype(np.float32).reshape(8, 128).T),
        "ab2m": np.ascontiguousarray(
            inp["attn_b2"].astype(np.float32).reshape(8, 128).T),
        "w1a": bfc(sw1[0:1024]),
        "w1b": bfc(sw1[1024:2048]),
        "w1c": bfc(sw1[2048:3072]),
        "w2": bfc(inp["score_w2"]),
        "b2m": np.ascontiguousarray(
            inp["score_b2"].astype(np.float32).reshape(8, 128).T),
        "w3m": bfc(inp["score_w3"].reshape(8, 128).T),
        "w1d": bfc(sw1[3072:3092]),
        "wtT": bfc(inp["width_table"].T),
        "b1r": bfc(inp["score_b1"].reshape(1, HID)),
        "iotaW": np.arange(K_WIN, dtype=np.float32).reshape(1, -1),
        "ident": np.eye(128, dtype=np.float32).astype(bf16),
    }

    states = inp["states"].astype(np.float32)
    embeds = inp["embeds"].astype(np.float32)
    in_maps = []
    for c in range(N_CORES):
        cb = int(plan["core_base"][c])
        stl = np.zeros((T_cap, D), np.float32)
        eml = np.zeros((T_cap, D), np.float32)
        hi = min(T, cb + T_cap)
        stl[: hi - cb] = states[cb:hi]
        eml[: hi - cb] = embeds[cb:hi]
        m = dict(shared)
        m["statesT"] = np.ascontiguousarray(stl.T).astype(bf16)
        m["statesTf"] = np.ascontiguousarray(stl.T).astype(f8e4)
        m["embedsT"] = np.ascontiguousarray(eml.T).astype(bf16)
        d = plan["d"][c].astype(np.float32)
        dl = plan["dl"][c].astype(np.float32)
        ln = plan["ln"][c].astype(np.int64)
        m["dmat"] = np.ascontiguousarray(d.reshape(G, 128).T)
        m["demat"] = np.ascontiguousarray(dl.reshape(G, 128).T)
        # host-built one-hot gather matrices, [token, span] layout
        NBLK = sum(kcs)
        boff = np.cumsum([0] + kcs)
        di = plan["d"][c].astype(np.int64).reshape(G, 128)
        dei = plan["dl"][c].astype(np.int64).reshape(G, 128)
        ohs = np.zeros((128, NBLK * 128), np.float32)
        ohe = np.zeros((128, NBLK * 128), np.float32)
        cols = np.arange(128)
        for g in range(G):
            for kk in range(kcs[g]):
                c0 = (boff[g] + kk) * 128
                for arr, idx in ((ohs, di[g]), (ohe, dei[g])):
                    r = idx - kk * 128
                    sel = (r >= 0) & (r < 128)
                    arr[r[sel], c0 + cols[sel]] = 1.0
        m["ohs"] = ohs.astype(bf16)
        m["ohe"] = ohe.astype(bf16)
        ohl = np.zeros((128, C), np.float32)
        ohl[ln, np.arange(C)] = 1.0
        m["ohl"] = ohl.astype(bf16)
        in_maps.append(m)

    return nc, in_maps, plan


def kernel(**inputs):
    global LAST_EXEC_NS
    from concourse.bass_utils import run_bass_kernel_spmd

    nc, in_maps, plan = _prepare(inputs)
    _install_ntff_shim()
    res = run_bass_kernel_spmd(nc, in_maps, list(range(N_CORES)), trace=TRACE)
    LAST_EXEC_NS = res.exec_time_ns

    out = np.empty(NSPAN, np.float32)
    for c in range(N_CORES):
        out[plan["order"][c * C : (c + 1) * C]] = np.asarray(
            res.results[c]["scores"]).reshape(-1)
    return out.reshape(NSPAN, 1)

